# revision 1
# baseline (speedup 1.0000x reference)
"""Causal self-attention (B=4, T=2048, C=1024, 16 heads) on 8 trn2 NeuronCores.

Sharding: core c -> (batch b = c//2, head-half hh = c%2). Each core computes
one batch x 8 heads: QKV column-parallel + out-proj row-parallel (Megatron);
the host sums the two partial outputs per batch. No collectives.

Self-contained: hardcodes shapes; builds/compiles the Bass program once per
process and runs it via run_bass_kernel_spmd on cores 0-7.
"""

import numpy as np
import ml_dtypes

B, T, C = 4, 2048, 1024
N_HEAD = 16
D = 64          # head dim
NHC = 8         # heads per core
CC = 512        # channels per core (NHC * D)
KO = 8          # contraction chunks of 128 over C
TM = 16         # t chunks of 128

_NC = None          # cached compiled Bass program
LAST_RESULTS = None  # BassKernelResults of the last run (for test harness)


def _off(i):
    # start offset of score tile i inside the packed pT tensor
    return 2048 * i - 64 * i * (i - 1)


PT_LEN = _off(16)  # 17408


def build_nc():
    import concourse.bacc as bacc
    import concourse.mybir as mybir
    import concourse.tile as tile
    from concourse.masks import make_upper_triangular
    from contextlib import ExitStack

    bf16 = mybir.dt.bfloat16
    f32 = mybir.dt.float32
    EXP = mybir.ActivationFunctionType.Exp

    nc = bacc.Bacc("TRN2", target_bir_lowering=False, debug=False)

    xT = nc.dram_tensor("xT", [C, T], bf16, kind="ExternalInput")
    wq = nc.dram_tensor("wqT", [C, CC], bf16, kind="ExternalInput")
    wk = nc.dram_tensor("wkT", [C, CC], bf16, kind="ExternalInput")
    wv = nc.dram_tensor("wvT", [C, CC], bf16, kind="ExternalInput")
    wp = nc.dram_tensor("wpT", [CC, C], bf16, kind="ExternalInput")
    out = nc.dram_tensor("out", [T, C], f32, kind="ExternalOutput")

    with tile.TileContext(nc) as tc, ExitStack() as ctx:
        const = ctx.enter_context(tc.tile_pool(name="const", bufs=1))
        # keep-mask for the diagonal 128x128 block of p.T tiles: 1 where tq>=tk
        mask_sb = const.tile([128, 128], bf16)
        make_upper_triangular(nc, mask_sb[:], val=1.0, diag=True)
        ones_f32 = const.tile([128, 128], f32)
        nc.vector.memset(ones_f32[:], 1.0)

        wts = ctx.enter_context(tc.tile_pool(name="wts", bufs=1))
        xT_sb = wts.tile([128, KO, T], bf16)
        wv_sb = wts.tile([128, KO, CC], bf16)
        wp_sb = wts.tile([128, 8, C], bf16)

        for k in range(KO):
            nc.sync.dma_start(
                xT_sb[:, k], xT.rearrange("(ko p) t -> p ko t", p=128)[:, k]
            )
        nc.sync.dma_start(wv_sb[:], wv.rearrange("(ko p) d -> p ko d", p=128))
        nc.sync.dma_start(
            wp_sb[64:128], wp.rearrange("(l p) e -> p l e", p=64)
        )

        data = ctx.enter_context(tc.tile_pool(name="data", bufs=1))
        qT_sb = data.tile([128, 4, T], bf16)
        kT_sb = data.tile([128, 4, T], bf16)
        # v2[p, ti, l, 0:128] = [ones(64) | v_l(64)] for every head l, so y
        # lands at psum rows 64:128 and the rowsum at rows 0:64 (the only
        # partition-base the HW custom ops handle correctly)
        v2_sb = data.tile([128, TM, 8, 128], bf16)
        yTn_sb = data.tile([128, 8, T], bf16)
        pT = data.tile([128, PT_LEN], bf16)

        nc.vector.memset(v2_sb[:, :, :, 0:64], 1.0)

        proj_ps = ctx.enter_context(
            tc.tile_pool(name="proj_ps", bufs=2, space="PSUM")
        )
        sT_ps = ctx.enter_context(tc.tile_pool(name="sT_ps", bufs=2, space="PSUM"))
        yT_ps = ctx.enter_context(tc.tile_pool(name="yT_ps", bufs=2, space="PSUM"))

        norm = ctx.enter_context(tc.tile_pool(name="norm", bufs=2))
        scrp = ctx.enter_context(tc.tile_pool(name="scrp", bufs=2))
        ost = ctx.enter_context(tc.tile_pool(name="ost", bufs=2))
        wqk = ctx.enter_context(tc.tile_pool(name="wqk", bufs=1))

        # ---- v projection: v[t, d] = x @ Wv^T  (lhsT = xT chunk, rhs = wvT) ----
        for mt in range(TM):
            ps = proj_ps.tile([128, 512], f32, name=f"psv{mt}", tag="proj")
            for k in range(KO):
                nc.tensor.matmul(
                    ps[:],
                    lhsT=xT_sb[:, k, 128 * mt : 128 * mt + 128],
                    rhs=wv_sb[:, k],
                    start=(k == 0),
                    stop=(k == KO - 1),
                )
            # scatter per-head 64-col blocks into v2 (even l -> cols 0:64,
            # odd l -> cols 64:128); ScalarE copies (ACT idle in this phase)
            psv = ps.rearrange("p (l c) -> p l c", c=64)
            nc.vector.tensor_copy(v2_sb[:, mt, :, 64:128], psv[:])

        def qk_proj(m):
            for (w_dram, o_sb, tg) in ((wq, qT_sb, "wqm"), (wk, kT_sb, "wkm")):
                wm = wqk.tile([128, KO, 128], bf16, name=f"{tg}{m}", tag=tg)
                nc.sync.dma_start(
                    wm[:],
                    w_dram.rearrange("(ko p) d -> p ko d", p=128)[
                        :, :, 128 * m : 128 * m + 128
                    ],
                )
                for n in range(4):
                    ps = proj_ps.tile([128, 512], f32, name=f"psqk{m}{n}", tag="proj")
                    for k in range(KO):
                        nc.tensor.matmul(
                            ps[:],
                            lhsT=wm[:, k, :],
                            rhs=xT_sb[:, k, 512 * n : 512 * n + 512],
                            start=(k == 0),
                            stop=(k == KO - 1),
                        )
                    nc.vector.tensor_copy(o_sb[:, m, 512 * n : 512 * n + 512], ps[:])

        def attention(h):
            hm, hp = h // 2, h % 2
            pb = 64 * hp  # partition base of this head's d-rows in qT/kT
            # ---- scores s.T(i) = k_i @ q^T (trimmed to tq >= 128*i) + exp ----
            for i in range(16):
                off = _off(i)
                W = T - 128 * i
                for c0 in range(0, W, 1024):
                    Wc = min(1024, W - c0)
                    st = sT_ps.tile([128, 1024], f32, name=f"st{h}_{i}", tag="sT")
                    for s0 in range(0, Wc, 512):
                        Ws = min(512, Wc - s0)
                        nc.tensor.matmul(
                            st[:, s0 : s0 + Ws],
                            lhsT=kT_sb[pb : pb + 64, hm, 128 * i : 128 * i + 128],
                            rhs=qT_sb[pb : pb + 64, hm, 128 * i + c0 + s0 :][:, :Ws],
                            start=True,
                            stop=True,
                        )
                    nc.scalar.activation(
                        pT[:, off + c0 : off + c0 + Wc], st[:, :Wc], EXP, scale=0.125
                    )
                # causal mask inside the diagonal 128x128 block
                nc.vector.tensor_mul(
                    pT[:, off : off + 128], pT[:, off : off + 128], mask_sb[:]
                )
            # ---- y.T spans: [y | r]^T = v_aug^T @ p.T, accumulated over tk ----
            for j in range(4):
                yt = yT_ps.tile([128, 512], f32, name=f"yt{h}_{j}", tag="yT")
                ilim = 4 * j + 4
                for i in range(ilim):
                    o = 512 * j - 128 * i
                    lhsT = v2_sb[:, i, h, :]
                    if o >= 0:
                        nc.tensor.matmul(
                            yt[:],
                            lhsT=lhsT,
                            rhs=pT[:, _off(i) + o : _off(i) + o + 512],
                            start=(i == 0),
                            stop=(i == ilim - 1),
                        )
                    else:
                        w = 512 + o
                        nc.tensor.matmul(
                            yt[:, -o:512],
                            lhsT=lhsT,
                            rhs=pT[:, _off(i) : _off(i) + w],
                            start=False,
                            stop=(i == ilim - 1),
                        )
                # normalize: y rows 64:128, rowsum replicated at rows 0:64
                yr0, rr0 = 64, 0
                ri = norm.tile([128, 512], f32, name=f"ri{h}{j}", tag="ri")
                rb = norm.tile([128, 512], f32, name=f"rb{h}{j}", tag="rb")
                sc = scrp.tile([128, 512], f32, name=f"sc{h}{j}", tag="sc")
                nc.vector.reciprocal_approx_accurate(
                    out=ri[rr0 : rr0 + 64, :],
                    in_=yt[rr0 : rr0 + 64, :],
                    scratch=sc[rr0 : rr0 + 64, :],
                )
                # broadcast the reciprocal row to all partitions via a K=1
                # matmul (ones column x recip row), staged through PSUM
                rb_ps = yT_ps.tile([128, 512], f32, name=f"rbp{h}{j}", tag="yT")
                nc.tensor.matmul(
                    rb_ps[:],
                    lhsT=ones_f32[rr0 : rr0 + 1, :],
                    rhs=ri[rr0 : rr0 + 1, :],
                    start=True,
                    stop=True,
                )
                nc.vector.tensor_copy(rb[:], rb_ps[:])
                nc.vector.tensor_mul(
                    yTn_sb[64:128, h, 512 * j : 512 * j + 512],
                    yt[64:128, :],
                    rb[64:128, :],
                )

        for m in range(4):
            qk_proj(m)
            attention(2 * m)
            attention(2 * m + 1)

        # ---- out projection: out[t, e] = y @ Wp_sub^T (lhsT = yTn chunk) ----
        outr = out.rearrange("(mt p) e -> p mt e", p=128)
        for mt in range(TM):
            for n in range(2):
                ps = proj_ps.tile([128, 512], f32, name=f"pso{mt}{n}", tag="proj")
                for l in range(8):
                    nc.tensor.matmul(
                        ps[:],
                        lhsT=yTn_sb[64:128, l, 128 * mt : 128 * mt + 128],
                        rhs=wp_sb[64:128, l, 512 * n : 512 * n + 512],
                        start=(l == 0),
                        stop=(l == 7),
                    )
                o_sb = ost.tile([128, 512], f32, name=f"ost{mt}{n}", tag="ost")
                nc.scalar.copy(o_sb[:], ps[:])
                nc.sync.dma_start(outr[:, mt, 512 * n : 512 * n + 512], o_sb[:])

    nc.compile()
    return nc


def _get_nc():
    global _NC
    if _NC is None:
        _NC = build_nc()
    return _NC


def kernel(x, Wk, Wq, Wv, Wp, _trace=False):
    from concourse.bass_utils import run_bass_kernel_spmd

    global LAST_RESULTS
    bf16 = ml_dtypes.bfloat16
    x = np.asarray(x, dtype=np.float32)
    Wk = np.asarray(Wk, dtype=np.float32)
    Wq = np.asarray(Wq, dtype=np.float32)
    Wv = np.asarray(Wv, dtype=np.float32)
    Wp = np.asarray(Wp, dtype=np.float32)

    in_maps = []
    for c in range(8):
        b, hh = c // 2, c % 2
        cols = slice(CC * hh, CC * hh + CC)
        in_maps.append(
            {
                "xT": np.ascontiguousarray(x[b].T).astype(bf16),
                "wqT": np.ascontiguousarray(Wq[cols, :].T).astype(bf16),
                "wkT": np.ascontiguousarray(Wk[cols, :].T).astype(bf16),
                "wvT": np.ascontiguousarray(Wv[cols, :].T).astype(bf16),
                "wpT": np.ascontiguousarray(Wp[:, cols].T).astype(bf16),
            }
        )

    nc = _get_nc()
    res = run_bass_kernel_spmd(nc, in_maps, core_ids=list(range(8)), trace=_trace)
    LAST_RESULTS = res

    out = np.empty((B, T, C), dtype=np.float32)
    for b in range(B):
        out[b] = res.results[2 * b]["out"] + res.results[2 * b + 1]["out"]
    return out


if __name__ == "__main__":
    rng = np.random.default_rng(0)
    s = 1.0 / np.sqrt(C)
    inputs = {
        "x": rng.standard_normal((B, T, C), dtype=np.float32),
        "Wk": rng.standard_normal((C, C), dtype=np.float32) * s,
        "Wq": rng.standard_normal((C, C), dtype=np.float32) * s,
        "Wv": rng.standard_normal((C, C), dtype=np.float32) * s,
        "Wp": rng.standard_normal((C, C), dtype=np.float32) * s,
    }
    got = kernel(**inputs)

    # numpy reference
    def ref(x, Wk, Wq, Wv, Wp):
        def heads(w):
            return (
                np.einsum("btc,ec->bte", x, w)
                .reshape(B, T, N_HEAD, D)
                .transpose(0, 2, 1, 3)
            )

        k, q, v = heads(Wk), heads(Wq), heads(Wv)
        att = np.einsum("bhqd,bhkd->bhqk", q, k) / np.sqrt(D)
        causal = np.tril(np.ones((T, T), dtype=bool))
        att = np.where(causal[None, None], att, -np.inf)
        att = att - att.max(axis=-1, keepdims=True)
        e = np.exp(att)
        p = e / e.sum(axis=-1, keepdims=True)
        y = np.einsum("bhqk,bhkd->bhqd", p, v)
        y = y.transpose(0, 2, 1, 3).reshape(B, T, C)
        return np.einsum("btc,ec->bte", y, Wp)

    want = ref(**{k: v.astype(np.float64) for k, v in inputs.items()}).astype(
        np.float32
    )
    rel = np.linalg.norm(got - want) / np.linalg.norm(want)
    print("rel l2 err:", rel)
    print("max abs err:", np.abs(got - want).max(), "ref absmax:", np.abs(want).max())



# revision 4
# speedup vs baseline: 1.7044x; 1.7044x over previous
"""Causal self-attention (B=4, T=2048, C=1024, 16 heads) on 8 trn2 NeuronCores.

Sharding: core c -> (batch b = c//2, head-half hh = c%2). Each core computes
one batch x 8 heads (4 head PAIRS): QKV column-parallel + out-proj
row-parallel (Megatron); the host sums the two partial outputs per batch.

Software-pipelined emission (engines execute their streams in order, so the
emission order IS the schedule):

  qk_proj(0)                      n-descending so scores can start early
  S(0)   scores+exp pair 0        v-proj + qk_proj(1) chains interleaved
  for m in 0..3:
      for j in 3..0:
          Y(m, j)                 y chains (j, hh=0/1) + normalize
          S(m+1, group j)         score chunks for i in 4j+3..4j -- these pT
                                  regions were freed by Y(m, j) just above
          [m==3: out-proj mt group 4j..4j+3 instead]
      [qk_proj(m+2) chains interleaved into the S(m+1) chunks]

Other key points vs the naive version:
  - score matmuls row-packed: head pair at array row groups 0:64 / 64:128
    (tile_position auto-derived) -> concurrent on HW
  - one [128, 2, 512] f32 PSUM score tile (bufs=2) per chunk; ONE exp
    activation covers both heads
  - normalization: reciprocal of rowsum rows 0:64 multiplied onto y rows
    64:128 with offset partition bases (no broadcast matmul)
  - yTn packed per PAIR across 128 partitions -> out-projection K=128
"""

import numpy as np
import ml_dtypes

B, T, C = 4, 2048, 1024
N_HEAD = 16
D = 64          # head dim
NHC = 8         # heads per core
CC = 512        # channels per core (NHC * D)
KO = 8          # contraction chunks of 128 over C
TM = 16         # t chunks of 128

_NC = None          # cached compiled Bass program
LAST_RESULTS = None  # BassKernelResults of the last run (for test harness)


# pT stores score regions in DESCENDING i order (emission order), so the
# exp stream is contiguous and can be cut into uniform 512-col chunks.
# region i occupies [_off(i), _off(i) + 2048 - 128 i)
_OFF2 = [0] * 17
for _i in range(14, -1, -1):
    _OFF2[_i] = _OFF2[_i + 1] + (2048 - 128 * (_i + 1))


def _off(i):
    return _OFF2[i]


PT_LEN = _OFF2[0] + 2048  # 17408


def build_nc():
    import concourse.bacc as bacc
    import concourse.mybir as mybir
    import concourse.tile as tile
    from concourse.masks import make_upper_triangular
    from contextlib import ExitStack

    bf16 = mybir.dt.bfloat16
    f32 = mybir.dt.float32
    EXP = mybir.ActivationFunctionType.Exp

    nc = bacc.Bacc("TRN2", target_bir_lowering=False, debug=False)

    xT = nc.dram_tensor("xT", [C, T], bf16, kind="ExternalInput")
    wq = nc.dram_tensor("wqT", [C, CC], bf16, kind="ExternalInput")
    wk = nc.dram_tensor("wkT", [C, CC], bf16, kind="ExternalInput")
    wv = nc.dram_tensor("wvT", [C, CC], bf16, kind="ExternalInput")
    wp = nc.dram_tensor("wpT", [CC, C], bf16, kind="ExternalInput")
    # partial outputs stored bf16 (halves store traffic); host sums in f32
    out = nc.dram_tensor("out", [T, C], bf16, kind="ExternalOutput")

    with tile.TileContext(nc) as tc, ExitStack() as ctx:
        const = ctx.enter_context(tc.tile_pool(name="const", bufs=1))
        # keep-mask for the diagonal 128x128 block of p.T tiles: 1 where tq>=tk
        mask_sb = const.tile([128, 128], bf16)
        make_upper_triangular(nc, mask_sb[:], val=1.0, diag=True)
        # touch Exp once at t=0 so the ACT table set loads during the
        # DMA lead-in instead of stalling the first real exp
        warm_sb = const.tile([1, 8], f32)
        nc.vector.memset(warm_sb[:], 0.0)
        nc.scalar.activation(warm_sb[:], warm_sb[:], EXP, scale=1.0)

        wts = ctx.enter_context(tc.tile_pool(name="wts", bufs=1))
        xT_sb = wts.tile([128, KO, T], bf16)
        wv_sb = wts.tile([128, KO, CC], bf16)
        # wp packed by head pair: partition p<64 = even head dims, p>=64 = odd
        wp_sb = wts.tile([128, 4, C], bf16)

        data = ctx.enter_context(tc.tile_pool(name="data", bufs=1))
        # v2[p, ti, l, 0:128] = [ones(64) | v_l(64)] for every head l, so y
        # lands at psum rows 64:128 and the rowsum at rows 0:64 (recip custom
        # op needs partition base 0)
        v2_sb = data.tile([128, TM, 8, 128], bf16)
        yTn_sb = data.tile([128, 4, T], bf16)   # per pair: even head rows 0:64
        pT = data.tile([128, 2, PT_LEN], bf16)  # per-pair A|B halves

        qk = ctx.enter_context(tc.tile_pool(name="qk", bufs=2))
        wqk = ctx.enter_context(tc.tile_pool(name="wqk", bufs=2))
        norm = ctx.enter_context(tc.tile_pool(name="norm", bufs=2))
        ost = ctx.enter_context(tc.tile_pool(name="ost", bufs=8))

        proj_ps = ctx.enter_context(
            tc.tile_pool(name="proj_ps", bufs=2, space="PSUM")
        )
        sT_ps = ctx.enter_context(tc.tile_pool(name="sT_ps", bufs=2, space="PSUM"))
        yT_ps = ctx.enter_context(tc.tile_pool(name="yT_ps", bufs=2, space="PSUM"))

        # ---------------- DMA loads ----------------
        # Loads are spread round-robin over all four DGE queues -- a single
        # queue serializes at ~0.6us/transfer. qk0 weights first (first score
        # work depends on them), then x in (n, k) chunks with n DESCENDING:
        # the first qk chains (n=3) only need the last t-quarter.
        qrr = (nc.sync, nc.scalar, nc.gpsimd)
        wm0 = {}
        for qi, (w_dram, tg) in enumerate(((wq, "q"), (wk, "k"))):
            wm = wqk.tile([128, KO, 128], bf16, name=f"w{tg}0", tag=f"w{tg}")
            qrr[qi].dma_start(
                wm[:], w_dram.rearrange("(ko p) d -> p ko d", p=128)[:, :, 0:128]
            )
            wm0[tg] = wm
        xTr = xT.rearrange("(ko p) t -> p ko t", p=128)
        di = 2
        for n in range(3, -1, -1):
            for k in range(KO):
                qrr[di % 3].dma_start(
                    xT_sb[:, k, 512 * n : 512 * n + 512],
                    xTr[:, k, 512 * n : 512 * n + 512],
                )
                di += 1
        nc.scalar.dma_start(wv_sb[:], wv.rearrange("(ko p) d -> p ko d", p=128))
        nc.gpsimd.dma_start(wp_sb[:], wp.rearrange("(pr p) e -> p pr e", p=128))

        nc.gpsimd.memset(v2_sb[:, :, :, 0:64], 1.0)

        # ---------------- chain emitters ----------------
        def qk_chain(o_sb, wm, n):
            """One q/k projection chain: d-chunk x 512 t-columns."""
            ps = proj_ps.tile([128, 512], f32, name=f"ps{o_sb.name}{n}", tag="proj")
            for k in range(KO):
                nc.tensor.matmul(
                    ps[:],
                    lhsT=wm[:, k, :],
                    rhs=xT_sb[:, k, 512 * n : 512 * n + 512],
                    start=(k == 0),
                    stop=(k == KO - 1),
                )
            nc.vector.tensor_copy(o_sb[:, 512 * n : 512 * n + 512], ps[:])

        def v_chain(mt):
            """v[t-chunk] = x @ Wv^T, scattered into v2 as [ones | v_l]."""
            ps = proj_ps.tile([128, 512], f32, name=f"psv{mt}", tag="proj")
            for k in range(KO):
                nc.tensor.matmul(
                    ps[:],
                    lhsT=xT_sb[:, k, 128 * mt : 128 * mt + 128],
                    rhs=wv_sb[:, k],
                    start=(k == 0),
                    stop=(k == KO - 1),
                )
            psv = ps.rearrange("p (l c) -> p l c", c=64)
            nc.vector.tensor_copy(v2_sb[:, mt, :, 64:128], psv[:])

        def make_qk_filler(m):
            """Return (q_sb, k_sb, [chain closures]) for pair m's projections.
            Chains n-descending: scores consume high-t columns first."""
            tiles = {}
            chains = []
            for (w_dram, tg) in ((wq, "q"), (wk, "k")):
                if m == 0:
                    wm = wm0[tg]
                else:
                    wm = wqk.tile([128, KO, 128], bf16, name=f"w{tg}{m}",
                                  tag=f"w{tg}")
                o_sb = qk.tile([128, T], bf16, name=f"{tg}{m}", tag=tg)
                tiles[tg] = (o_sb, wm)
            def dma(m=m):
                for (w_dram, tg) in ((wq, "q"), (wk, "k")):
                    if m > 0:
                        nc.sync.dma_start(
                            tiles[tg][1][:],
                            w_dram.rearrange("(ko p) d -> p ko d", p=128)[
                                :, :, 128 * m : 128 * m + 128
                            ],
                        )
            for n in range(3, -1, -1):
                for tg in ("q", "k"):
                    o_sb, wm = tiles[tg]
                    chains.append(lambda o=o_sb, w=wm, n=n: qk_chain(o, w, n))
            return tiles["q"][0], tiles["k"][0], dma, chains

        filler = []  # queue of chain closures, popped on a cadence

        # stream chunk ci covers pT positions [512 ci, 512 ci + 512); it is
        # assigned to the group of its LAST (lowest-i, latest-freed) region
        def _chunk_regions(ci):
            p0 = 512 * ci
            segs = []
            for i in range(15, -1, -1):
                a = max(p0, _OFF2[i])
                b = min(p0 + 512, _OFF2[i] + 2048 - 128 * i)
                if a < b:
                    segs.append((i, a, b))
            return segs

        _GROUP_CHUNKS = {3: [], 2: [], 1: [], 0: []}
        for _ci in range(PT_LEN // 512):
            _GROUP_CHUNKS[_chunk_regions(_ci)[-1][0] // 4].append(_ci)

        def score_group(m, q_sb, k_sb, g, fill_quota):
            """Scores+exp for pair m: the uniform 512-col stream chunks whose
            pT regions are freed by Y(m-1, j=g); `fill_quota` filler chains
            interleaved evenly."""
            chunks = _GROUP_CHUNKS[g]
            nch = len(chunks)
            popped = 0
            for cn, ci in enumerate(chunks):
                p0 = 512 * ci
                st = sT_ps.tile([128, 2, 512], f32, name=f"st{m}_{ci}",
                                tag="st")
                for hh in (0, 1):
                    pb = 64 * hh
                    for (i, a, b) in _chunk_regions(ci):
                        q0c = 128 * i + (a - _OFF2[i])
                        nc.tensor.matmul(
                            st[:, hh, a - p0 : b - p0],
                            lhsT=k_sb[pb : pb + 64, 128 * i : 128 * i + 128],
                            rhs=q_sb[pb : pb + 64, q0c : q0c + (b - a)],
                            start=True,
                            stop=True,
                        )
                nc.scalar.activation(
                    pT[:, :, p0 : p0 + 512],
                    st[:, :, :],
                    EXP,
                    scale=0.125,
                )
                # causal mask for any diagonal 128x128 block ending in chunk
                for (i, a, b) in _chunk_regions(ci):
                    if a <= _OFF2[i] + 127 < b:
                        for hh in (0, 1):
                            nc.vector.tensor_mul(
                                pT[:, hh, _OFF2[i] : _OFF2[i] + 128],
                                pT[:, hh, _OFF2[i] : _OFF2[i] + 128],
                                mask_sb[:],
                            )
                want = (fill_quota * (cn + 1)) // nch
                while popped < want and filler:
                    filler.pop(0)()
                    popped += 1

        def y_group(m, j):
            """y chains (j, hh=0) and (j, hh=1), MM-interleaved, + norms.

            i DESCENDING and hh-interleaved: a pT region is read (and thus
            freed for the next pair's exp writes) after just two matmuls.
            The partial-width diagonal matmul legally opens each group:
            start=True clears has_written for the whole bank, later
            full-width matmuls overwrite where the bit is clear."""
            ilim = 4 * j + 4
            yts = [yT_ps.tile([128, 512], f32, name=f"yt{2*m+hh}_{j}", tag="yt")
                   for hh in (0, 1)]
            for i in range(ilim - 1, -1, -1):
                o = 512 * j - 128 * i
                for hh in (0, 1):
                    yt = yts[hh]
                    lhsT = v2_sb[:, i, 2 * m + hh, :]
                    if o >= 0:
                        nc.tensor.matmul(
                            yt[:],
                            lhsT=lhsT,
                            rhs=pT[:, hh, _off(i) + o : _off(i) + o + 512],
                            start=(i == ilim - 1),
                            stop=(i == 0),
                        )
                    else:
                        w = 512 + o
                        nc.tensor.matmul(
                            yt[:, -o:512],
                            lhsT=lhsT,
                            rhs=pT[:, hh, _off(i) : _off(i) + w],
                            start=(i == ilim - 1),
                            stop=False,
                        )
            for hh in (0, 1):
                h = 2 * m + hh
                dst = slice(64 * hh, 64 * hh + 64)  # yTn partition rows
                yt = yts[hh]
                # rowsum r sits replicated on psum rows 0:64, y on 64:128.
                # approx_fast (~12 bits) is plenty: softmax weights are bf16.
                ri = norm.tile([64, 512], f32, name=f"ri{h}{j}", tag="ri")
                nc.vector.reciprocal_approx_fast(out=ri[:], in_=yt[0:64, :])
                nc.vector.tensor_mul(
                    yTn_sb[dst, m, 512 * j : 512 * j + 512],
                    yt[64:128, :],
                    ri[:],
                )

        outr = out.rearrange("(mt p) e -> p mt e", p=128)

        def outproj_group(j):
            """out[t, e] chains for mt in 4j..4j+3 (needs Y(3, j) done).
            PSUM alternates between the proj pool and the (now idle) score
            pool; copies alternate DVE/ACT; stores alternate DMA queues."""
            for gi, (mt, n) in enumerate(
                (mt, n) for mt in range(4 * j, 4 * j + 4) for n in range(2)
            ):
                if gi % 2 == 0:
                    ps3 = sT_ps.tile([128, 2, 512], f32, name=f"pso{mt}{n}",
                                     tag="st")
                    ps = ps3[:, 0, :]
                else:
                    ps = proj_ps.tile([128, 512], f32, name=f"pso{mt}{n}",
                                      tag="proj")[:]
                for pr in range(4):
                    nc.tensor.matmul(
                        ps,
                        lhsT=yTn_sb[:, pr, 128 * mt : 128 * mt + 128],
                        rhs=wp_sb[:, pr, 512 * n : 512 * n + 512],
                        start=(pr == 0),
                        stop=(pr == 3),
                    )
                o_sb = ost.tile([128, 512], bf16, name=f"ost{mt}{n}",
                                tag="ost")
                if gi % 2 == 0:
                    nc.vector.tensor_copy(o_sb[:], ps)
                else:
                    nc.scalar.copy(o_sb[:], ps)
                dq = (nc.sync, nc.scalar, nc.gpsimd)[gi % 3]
                dq.dma_start(outr[:, mt, 512 * n : 512 * n + 512], o_sb[:])

        # ---------------- pipeline ----------------
        # prologue: qk0 chains interleaved with the score groups they feed
        # (group g only needs the n=g chains), v-proj + qk_proj(1) as filler
        q0, k0, dma0, qk0_chains = make_qk_filler(0)
        q1, k1, dma1, qk1_chains = make_qk_filler(1)
        dma1()
        filler.extend(qk1_chains)
        filler.extend([lambda mt=mt: v_chain(mt) for mt in range(TM)])
        qs = {0: (q0, k0), 1: (q1, k1)}
        for ch in qk0_chains:
            ch()
        for g in range(3, -1, -1):
            score_group(0, q0, k0, g, fill_quota=6)

        for m in range(4):
            if m < 2:
                qn, kn, dman, qkn_chains = make_qk_filler(m + 2)
                dman()
                filler.extend(qkn_chains)
                qs[m + 2] = (qn, kn)
            for j in range(3, -1, -1):
                y_group(m, j)
                if m < 3:
                    qn, kn = qs[m + 1]
                    score_group(m + 1, qn, kn, g=j, fill_quota=2)
                else:
                    outproj_group(j)
        # drain any leftover filler (shouldn't happen)
        while filler:
            filler.pop(0)()

    nc.compile()
    return nc


def _get_nc():
    global _NC
    if _NC is None:
        _NC = build_nc()
    return _NC


def kernel(x, Wk, Wq, Wv, Wp, _trace=False):
    from concourse.bass_utils import run_bass_kernel_spmd

    global LAST_RESULTS
    bf16 = ml_dtypes.bfloat16
    x = np.asarray(x, dtype=np.float32)
    Wk = np.asarray(Wk, dtype=np.float32)
    Wq = np.asarray(Wq, dtype=np.float32)
    Wv = np.asarray(Wv, dtype=np.float32)
    Wp = np.asarray(Wp, dtype=np.float32)

    in_maps = []
    for c in range(8):
        b, hh = c // 2, c % 2
        cols = slice(CC * hh, CC * hh + CC)
        in_maps.append(
            {
                "xT": np.ascontiguousarray(x[b].T).astype(bf16),
                "wqT": np.ascontiguousarray(Wq[cols, :].T).astype(bf16),
                "wkT": np.ascontiguousarray(Wk[cols, :].T).astype(bf16),
                "wvT": np.ascontiguousarray(Wv[cols, :].T).astype(bf16),
                "wpT": np.ascontiguousarray(Wp[:, cols].T).astype(bf16),
            }
        )

    nc = _get_nc()
    res = run_bass_kernel_spmd(nc, in_maps, core_ids=list(range(8)), trace=_trace)
    LAST_RESULTS = res

    out = np.empty((B, T, C), dtype=np.float32)
    for b in range(B):
        out[b] = res.results[2 * b]["out"].astype(np.float32) + \
            res.results[2 * b + 1]["out"].astype(np.float32)
    return out


if __name__ == "__main__":
    rng = np.random.default_rng(0)
    s = 1.0 / np.sqrt(C)
    inputs = {
        "x": rng.standard_normal((B, T, C), dtype=np.float32),
        "Wk": rng.standard_normal((C, C), dtype=np.float32) * s,
        "Wq": rng.standard_normal((C, C), dtype=np.float32) * s,
        "Wv": rng.standard_normal((C, C), dtype=np.float32) * s,
        "Wp": rng.standard_normal((C, C), dtype=np.float32) * s,
    }
    got = kernel(**inputs)

    def ref(x, Wk, Wq, Wv, Wp):
        def heads(w):
            return (
                np.einsum("btc,ec->bte", x, w)
                .reshape(B, T, N_HEAD, D)
                .transpose(0, 2, 1, 3)
            )

        k, q, v = heads(Wk), heads(Wq), heads(Wv)
        att = np.einsum("bhqd,bhkd->bhqk", q, k) / np.sqrt(D)
        causal = np.tril(np.ones((T, T), dtype=bool))
        att = np.where(causal[None, None], att, -np.inf)
        att = att - att.max(axis=-1, keepdims=True)
        e = np.exp(att)
        p = e / e.sum(axis=-1, keepdims=True)
        y = np.einsum("bhqk,bhkd->bhqd", p, v)
        y = y.transpose(0, 2, 1, 3).reshape(B, T, C)
        return np.einsum("btc,ec->bte", y, Wp)

    want = ref(**{k: v.astype(np.float64) for k, v in inputs.items()}).astype(
        np.float32
    )
    rel = np.linalg.norm(got - want) / np.linalg.norm(want)
    print("rel l2 err:", rel)
    print("max abs err:", np.abs(got - want).max(), "ref absmax:", np.abs(want).max())
